# revision 8
# baseline (speedup 1.0000x reference)
"""BitNet attention (GQA, 32 q-heads / 8 kv-heads, hidden 4096, seq 2048) on 8
Trainium2 NeuronCores.

Sharding: tensor-parallel over heads. Core i computes q-heads 4i..4i+3 and
kv-head i (N_REP=4, so the 4 q-heads of core i attend exactly to kv-head i),
plus the o_proj contribution of its 512 hidden columns; the host sums the 8
partial o_proj outputs.

Device-side layout choices (per core):
  - x is passed transposed as xT [4096, 2048] bf16; weights are binarized on
    the host (sign(W), exactly representable in bf16) and passed transposed.
  - Q/K are produced feature-on-partition (Qt/Kt = [d, T]); V token-on-
    partition ([T, d]).
  - Scores are computed transposed, S.T[k, q] = Kt_tile^T @ Qt, so the exp
    output P.T[k, q] feeds directly as lhsT/rhs of the O.T and row-sum
    matmuls without any transposes.
  - softmax has no max-subtraction (scores are O(1) by construction: binary
    weights with per-row mean-abs rescale keep |scores| ~ 1).
  - The softmax denominator E[q] is computed as ones^T @ P.T on the PE; the
    division and the per-d v-scale are fused into one rank-1 matrix
    sv ⊗ (1/E) (outer-product matmul) applied on the O.T PSUM->SBUF copyout.
"""

import numpy as np
import ml_dtypes

import concourse.bass as bass
import concourse.mybir as mybir
import concourse.tile as tile
from concourse.vector_clock import ScopedClock
from concourse.bass_utils import run_bass_kernel_spmd

F32 = mybir.dt.float32
BF16 = mybir.dt.bfloat16

HIDDEN = 4096
T = 2048          # sequence length
N_CORES = 8
FQ = HIDDEN // N_CORES   # 512 q-features per core
H = 4                    # q heads per core
DH = 128                 # head dim
DC = HIDDEN // 128       # 32 contraction chunks
TQ = 4                   # token quarters (512 tokens each)
KT = T // 128            # 16 key tiles
QB = 4                   # query blocks of 512

TRACE = False            # set by test.py for profiling runs

_MAX_DRAIN_WAITS = 1
_MAX_INST_WAITS = 1


def _split_sync_waits(nc):
    """The walrus build in this container rejects instructions carrying more
    than one sync wait ("Too many sync wait commands"). Cap every instruction
    at _MAX_INST_WAITS waits; spill the excess onto InstNoOp instructions
    inserted immediately before on the same engine (engines are in-order, so
    the combined wait semantics are identical)."""
    counter = [0]

    def _mk_nop(engine, waits):
        counter[0] += 1
        nop = mybir.InstEventSemaphore(
            name=f"waitsplit_{counter[0]}", ins=[], outs=[]
        )
        nop.engine = engine
        nop.sync_info = mybir.SyncInfo(on_wait=list(waits), on_update=[])
        nc.register_instruction(nop, overwrite=True)
        return nop

    for bb in nc.main_func.blocks:
        insts = list(bb.instructions)
        out = []
        changed = False
        for ins in insts:
            si = ins.sync_info
            waits = list(si.on_wait or []) if si else []
            if len(waits) > _MAX_INST_WAITS:
                changed = True
                rest = waits[:-_MAX_INST_WAITS]
                for i in range(0, len(rest), _MAX_INST_WAITS):
                    out.append(_mk_nop(ins.engine, rest[i : i + _MAX_INST_WAITS]))
                ins.sync_info = mybir.SyncInfo(
                    on_wait=waits[-_MAX_INST_WAITS:],
                    on_update=list(si.on_update or []),
                )
            out.append(ins)
        if changed:
            bb.instructions = out


class _PatchedTileContext(tile.TileContext):
    """The walrus build in this container rejects CTRL (Drain) instructions
    carrying more than one sync wait. Tile's exit path puts one wait per live
    proc-sem on the final sync drain; spill the excess onto preceding
    sync-engine NOPs (same engine, in-order, semantics preserved)."""

    def _drain_and_barrier(self, tick_clock, wait_clock):
        nc = self.nc
        drain_inst = nc.sync.drain()
        wait_clock.add_sem_waits(
            drain_inst.ins, ScopedClock({None: tick_clock.global_clock})
        )
        ins = drain_inst.ins
        si = ins.sync_info
        waits = list(si.on_wait or []) if si else []
        updates = list(si.on_update or []) if si else []
        if len(waits) > _MAX_DRAIN_WAITS:
            ins.sync_info = mybir.SyncInfo(
                on_wait=waits[:_MAX_DRAIN_WAITS], on_update=updates
            )
            rest = waits[_MAX_DRAIN_WAITS:]
            for i in range(0, len(rest), _MAX_DRAIN_WAITS):
                nop = nc.sync.nop(nofuse=True, hint=f"dw{i}")
                nop.ins.sync_info = mybir.SyncInfo(
                    on_wait=rest[i : i + _MAX_DRAIN_WAITS], on_update=[]
                )
        nc.all_engine_barrier()
        assert self.sems is not None
        popped = nc._tile_sem_poison_stack.pop()
        assert popped is self._sem_poison
        nc.clear_and_free_semaphores(list(self.sems.allocated().values()))
        nc.all_engine_barrier()


def _build(split_waits=True):
    nc = bass.Bass()

    xt_d = nc.dram_tensor("xt", [HIDDEN, T], BF16, kind="ExternalInput")
    bqt_d = nc.dram_tensor("bqt", [HIDDEN, FQ], BF16, kind="ExternalInput")
    bkt_d = nc.dram_tensor("bkt", [HIDDEN, DH], BF16, kind="ExternalInput")
    bvt_d = nc.dram_tensor("bvt", [HIDDEN, DH], BF16, kind="ExternalInput")
    bot_d = nc.dram_tensor("bot", [FQ, HIDDEN], BF16, kind="ExternalInput")
    sq_d = nc.dram_tensor("sq", [H, DH, 1], F32, kind="ExternalInput")
    sk_d = nc.dram_tensor("sk", [DH, 1], F32, kind="ExternalInput")
    sv_d = nc.dram_tensor("sv", [1, DH], F32, kind="ExternalInput")
    ones_d = nc.dram_tensor("ones", [DH, 1], BF16, kind="ExternalInput")
    y_d = nc.dram_tensor("y", [T, HIDDEN], BF16, kind="ExternalOutput")

    with _PatchedTileContext(nc) as tc:
        with (
            tc.tile_pool(name="wq", bufs=DC) as wq,
            tc.tile_pool(name="wk", bufs=DC) as wk,
            tc.tile_pool(name="wv", bufs=DC) as wv,
            tc.tile_pool(name="xt", bufs=DC + 4) as xtp,
            tc.tile_pool(name="qt", bufs=H) as qtp,
            tc.tile_pool(name="kt", bufs=1) as ktp,
            tc.tile_pool(name="vv", bufs=TQ) as vvp,
            tc.tile_pool(name="pt", bufs=2 * KT) as ptp,
            tc.tile_pool(name="ot", bufs=H) as otp,
            tc.tile_pool(name="wo", bufs=8) as wop,
            tc.tile_pool(name="ys", bufs=4) as ysp,
            tc.tile_pool(name="misc", bufs=8) as misc,
            tc.tile_pool(name="psA", bufs=2, space="PSUM") as psA,
            tc.tile_pool(name="psS", bufs=2, space="PSUM") as psS,
            tc.tile_pool(name="psO", bufs=2, space="PSUM") as psO,
            tc.tile_pool(name="psE", bufs=2, space="PSUM") as psE,
        ):
            # --- constants / scales -------------------------------------
            sq_sb = [misc.tile([DH, 1], F32, tag=f"sq{f}", name=f"sq{f}") for f in range(H)]
            for f in range(H):
                nc.sync.dma_start(sq_sb[f][:], sq_d[f])
            sk_sb = misc.tile([DH, 1], F32, tag="sk")
            nc.sync.dma_start(sk_sb[:], sk_d[:])
            sv_sb = misc.tile([1, DH], F32, tag="sv")
            nc.sync.dma_start(sv_sb[:], sv_d[:])
            ones_sb = misc.tile([DH, 1], BF16, tag="ones")
            nc.sync.dma_start(ones_sb[:], ones_d[:])

            # --- weights -------------------------------------------------
            bqt_sb, bkt_sb, bvt_sb = [], [], []
            for dc in range(DC):
                t_ = wq.tile([128, FQ], BF16, tag="wq", name=f"wq{dc}")
                nc.sync.dma_start(t_[:], bqt_d[dc * 128 : (dc + 1) * 128, :])
                bqt_sb.append(t_)
                t_ = wk.tile([128, DH], BF16, tag="wk", name=f"wk{dc}")
                nc.sync.dma_start(t_[:], bkt_d[dc * 128 : (dc + 1) * 128, :])
                bkt_sb.append(t_)
                t_ = wv.tile([128, DH], BF16, tag="wv", name=f"wv{dc}")
                nc.sync.dma_start(t_[:], bvt_d[dc * 128 : (dc + 1) * 128, :])
                bvt_sb.append(t_)

            # --- persistent activation tiles ----------------------------
            qt_sb = [qtp.tile([DH, T], BF16, tag="qt", name=f"qt{f}") for f in range(H)]
            kt_sb = ktp.tile([DH, T], BF16, tag="kt")
            vv_sb = [vvp.tile([128, 512], BF16, tag="vv", name=f"vv{tq}") for tq in range(TQ)]
            ot_sb = [otp.tile([DH, T], BF16, tag="ot", name=f"ot{f}") for f in range(H)]

            # --- phase 1: q/k/v projections, one token-quarter at a time
            for tq in range(TQ):
                tsl = slice(tq * 512, (tq + 1) * 512)
                xt_sb = []
                for dc in range(DC):
                    t_ = xtp.tile([128, 512], BF16, tag="xt", name=f"xt{tq}_{dc}")
                    nc.sync.dma_start(t_[:], xt_d[dc * 128 : (dc + 1) * 128, tsl])
                    xt_sb.append(t_)

                # K projection: Kt[:, tq] += BkT_chunk^T @ xT_chunk
                ps = psA.tile([128, 512], F32, tag="psA")
                for dc in range(DC):
                    nc.tensor.matmul(
                        ps[:], bkt_sb[dc][:], xt_sb[dc][:],
                        start=(dc == 0), stop=(dc == DC - 1),
                    )
                nc.scalar.activation(
                    kt_sb[:, tsl], ps[:],
                    mybir.ActivationFunctionType.Copy, scale=sk_sb[:],
                )

                # V projection: V[tq][:, vt*128+d] (tokens on partitions)
                ps = psA.tile([128, 512], F32, tag="psA")
                for vt in range(4):
                    for dc in range(DC):
                        nc.tensor.matmul(
                            ps[:, vt * 128 : (vt + 1) * 128],
                            xt_sb[dc][:, vt * 128 : (vt + 1) * 128],
                            bvt_sb[dc][:],
                            start=(dc == 0), stop=(dc == DC - 1),
                        )
                nc.vector.tensor_copy(out=vv_sb[tq][:], in_=ps[:])

                # Q projection: Qt[f][:, tq] += BqT_chunk[:, f]^T @ xT_chunk
                for f in range(H):
                    ps = psA.tile([128, 512], F32, tag="psA")
                    for dc in range(DC):
                        nc.tensor.matmul(
                            ps[:],
                            bqt_sb[dc][:, f * 128 : (f + 1) * 128],
                            xt_sb[dc][:],
                            start=(dc == 0), stop=(dc == DC - 1),
                        )
                    nc.scalar.activation(
                        qt_sb[f][:, tsl], ps[:],
                        mybir.ActivationFunctionType.Copy, scale=sq_sb[f][:],
                    )

            # --- phase 2: attention per (head, query-block) --------------
            for h in range(H):
                for qb in range(QB):
                    qsl = slice(qb * 512, (qb + 1) * 512)
                    # scores (transposed) + exp
                    pt_sb = []
                    for kt in range(KT):
                        ps_s = psS.tile([128, 512], F32, tag="psS")
                        nc.tensor.matmul(
                            ps_s[:],
                            kt_sb[:, kt * 128 : (kt + 1) * 128],
                            qt_sb[h][:, qsl],
                            start=True, stop=True,
                        )
                        pt = ptp.tile([128, 512], BF16, tag="pt", name=f"pt{h}_{qb}_{kt}")
                        nc.scalar.activation(
                            pt[:], ps_s[:], mybir.ActivationFunctionType.Exp
                        )
                        pt_sb.append(pt)
                    # O.T = V^T @ P.T accumulated over key tiles; E = ones^T @ P.T
                    ps_o = psO.tile([128, 512], F32, tag="psO")
                    ps_e = psE.tile([1, 512], F32, tag="psE")
                    for kt in range(KT):
                        tqi, vti = divmod(kt, 4)
                        nc.tensor.matmul(
                            ps_o[:],
                            vv_sb[tqi][:, vti * 128 : (vti + 1) * 128],
                            pt_sb[kt][:],
                            start=(kt == 0), stop=(kt == KT - 1),
                        )
                        nc.tensor.matmul(
                            ps_e[:], ones_sb[:], pt_sb[kt][:],
                            start=(kt == 0), stop=(kt == KT - 1),
                        )
                    recip = misc.tile([1, 512], F32, tag="recip")
                    nc.vector.reciprocal(recip[:], ps_e[:])
                    ps_sc = psS.tile([128, 512], F32, tag="psS")
                    nc.tensor.matmul(
                        ps_sc[:], sv_sb[:], recip[:], start=True, stop=True
                    )
                    sc_sb = misc.tile([128, 512], F32, tag="sc", name=f"sc{h}_{qb}")
                    nc.scalar.activation(
                        sc_sb[:], ps_sc[:], mybir.ActivationFunctionType.Copy
                    )
                    nc.vector.tensor_tensor(
                        ot_sb[h][:, qsl], ps_o[:], sc_sb[:],
                        mybir.AluOpType.mult,
                    )

            # --- phase 3: o_proj partial --------------------------------
            for ob in range(8):
                osl = slice(ob * 512, (ob + 1) * 512)
                bot_sb = []
                for c in range(H):
                    t_ = wop.tile([128, 512], BF16, tag="wo", name=f"wo{ob}_{c}")
                    nc.sync.dma_start(t_[:], bot_d[c * 128 : (c + 1) * 128, osl])
                    bot_sb.append(t_)
                for tt in range(16):
                    ps_y = psA.tile([128, 512], F32, tag="psA")
                    for c in range(H):
                        nc.tensor.matmul(
                            ps_y[:],
                            ot_sb[c][:, tt * 128 : (tt + 1) * 128],
                            bot_sb[c][:],
                            start=(c == 0), stop=(c == H - 1),
                        )
                    ysb = ysp.tile([128, 512], BF16, tag="ys")
                    nc.any.tensor_copy(out=ysb[:], in_=ps_y[:])
                    nc.sync.dma_start(
                        y_d[tt * 128 : (tt + 1) * 128, osl], ysb[:]
                    )

    if split_waits:
        _split_sync_waits(nc)
    return nc


_NC_CACHE = None


def _get_nc():
    global _NC_CACHE
    if _NC_CACHE is None:
        _NC_CACHE = _build()
    return _NC_CACHE


def _binarize(w):
    """Match reference bitnet_linear: s = max(mean|W|_row, 1e-8) (>0), so
    sign(W/s) == sign(W). Returns (sign(W) as bf16, s as f32)."""
    w = np.asarray(w, np.float32)
    s = np.maximum(
        np.abs(w).mean(axis=1, dtype=np.float64).astype(np.float32), 1e-8
    )
    return np.sign(w).astype(ml_dtypes.bfloat16), s


def _make_in_maps(hidden_states, q_weight, q_scale, k_weight, k_scale,
                  v_weight, v_scale, o_weight, o_scale):
    hs = np.asarray(hidden_states, np.float32)
    b, t, hid = hs.shape
    assert (b, t, hid) == (1, T, HIDDEN)

    xT = np.ascontiguousarray(hs[0].T).astype(ml_dtypes.bfloat16)
    bq, s_q = _binarize(q_weight)
    bk, s_k = _binarize(k_weight)
    bv, s_v = _binarize(v_weight)
    bo, s_o = _binarize(o_weight)

    sq_full = s_q * np.asarray(q_scale, np.float32)            # [4096]
    sk_full = s_k * np.asarray(k_scale, np.float32) / np.sqrt(DH)  # [1024]
    sv_full = s_v * np.asarray(v_scale, np.float32)            # [1024]
    so_full = s_o * np.asarray(o_scale, np.float32)            # [4096]

    ones = np.ones((DH, 1), ml_dtypes.bfloat16)
    in_maps = []
    for i in range(N_CORES):
        fq = slice(FQ * i, FQ * (i + 1))
        fk = slice(DH * i, DH * (i + 1))
        in_maps.append({
            "xt": xT,
            "bqt": np.ascontiguousarray(bq[fq].T),
            "bkt": np.ascontiguousarray(bk[fk].T),
            "bvt": np.ascontiguousarray(bv[fk].T),
            "bot": np.ascontiguousarray(bo[:, fq].T),
            "sq": np.ascontiguousarray(
                sq_full[fq].reshape(H, DH, 1).astype(np.float32)
            ),
            "sk": np.ascontiguousarray(
                sk_full[fk].reshape(DH, 1).astype(np.float32)
            ),
            "sv": np.ascontiguousarray(
                sv_full[fk].reshape(1, DH).astype(np.float32)
            ),
            "ones": ones,
        })
    return in_maps, so_full


def kernel(**inputs):
    in_maps, so_full = _make_in_maps(**inputs)
    nc = _get_nc()
    res = run_bass_kernel_spmd(
        nc, in_maps, core_ids=list(range(N_CORES)), trace=TRACE
    )
    if TRACE:
        kernel.last_exec_time_ns = res.exec_time_ns
        kernel.last_mean_exec_time_ns = res.mean_exec_time_ns

    y = np.zeros((T, HIDDEN), np.float32)
    for i in range(N_CORES):
        y += res.results[i]["y"].astype(np.float32)
    y *= so_full[None, :]
    return y.reshape(1, T, HIDDEN)


# revision 11
# speedup vs baseline: 1.1771x; 1.1771x over previous
"""BitNet attention (GQA, 32 q-heads / 8 kv-heads, hidden 4096, seq 2048) on 8
Trainium2 NeuronCores.

Sharding: tensor-parallel over heads. Core i computes q-heads 4i..4i+3 and
kv-head i (N_REP=4, so the 4 q-heads of core i attend exactly to kv-head i),
plus the o_proj contribution of its 512 hidden columns; the host sums the 8
partial o_proj outputs.

Device-side layout choices (per core):
  - All matmul operands are bf16 (binary weights are exactly +-1/0 in bf16;
    fp32 matmul is 4x slower on the PE); PSUM accumulation is fp32.
  - x is passed transposed as xT; all streaming blocks are repacked on the
    host into partition-major contiguous layouts so each block is ONE DMA
    (descriptor-issue on the sync engine was a startup bottleneck).
  - Q/K are produced feature-on-partition (Qt/Kt = [d, T]); V token-on-
    partition ([T, d]).
  - Scores are computed transposed, S.T[k, q] = Kt_tile^T @ Qt, so the exp
    output P.T[k, q] feeds directly as lhsT/rhs of the O.T and row-sum
    matmuls without any transposes. Scores PSUM tiles span 2 banks (two key
    tiles) so each ACT exp op amortizes its ~352-cycle fixed overhead.
  - softmax has no max-subtraction (scores are O(1) by construction: binary
    weights with per-row mean-abs rescale keep |scores| ~ 1).
  - The softmax denominator E[q] is computed as ones^T @ P.T on the PE; the
    division and the per-d v-scale are fused into one rank-1 matrix
    sv (x) (1/E) (outer-product matmul) applied on the O.T PSUM->SBUF
    copyout.
"""

import numpy as np
import ml_dtypes

import concourse.bass as bass
import concourse.mybir as mybir
import concourse.tile as tile
from concourse.vector_clock import ScopedClock
from concourse.bass_utils import run_bass_kernel_spmd

F32 = mybir.dt.float32
BF16 = mybir.dt.bfloat16

HIDDEN = 4096
T = 2048          # sequence length
N_CORES = 8
FQ = HIDDEN // N_CORES   # 512 q-features per core
H = 4                    # q heads per core
DH = 128                 # head dim
DC = HIDDEN // 128       # 32 contraction chunks
HC = DC // 2             # 16 chunks per xt half
TQ = 4                   # token quarters (512 tokens each)
KT = T // 128            # 16 key tiles
QB = 4                   # query blocks of 512

TRACE = False            # set by test.py for profiling runs

_MAX_DRAIN_WAITS = 1
_MAX_INST_WAITS = 1


def _split_sync_waits(nc):
    """The walrus build in this container rejects instructions carrying more
    than one sync wait ("Too many sync wait commands"). Cap every instruction
    at _MAX_INST_WAITS waits; spill the excess onto InstEventSemaphore
    (standalone wait) instructions inserted immediately before on the same
    engine (engines are in-order, so combined wait semantics are identical)."""
    counter = [0]

    def _mk_wait(engine, waits):
        counter[0] += 1
        nop = mybir.InstEventSemaphore(
            name=f"waitsplit_{counter[0]}", ins=[], outs=[]
        )
        nop.engine = engine
        nop.sync_info = mybir.SyncInfo(on_wait=list(waits), on_update=[])
        nc.register_instruction(nop, overwrite=True)
        return nop

    for bb in nc.main_func.blocks:
        insts = list(bb.instructions)
        out = []
        changed = False
        for ins in insts:
            si = ins.sync_info
            waits = list(si.on_wait or []) if si else []
            if len(waits) > _MAX_INST_WAITS:
                changed = True
                rest = waits[:-_MAX_INST_WAITS]
                for i in range(0, len(rest), _MAX_INST_WAITS):
                    out.append(_mk_wait(ins.engine, rest[i : i + _MAX_INST_WAITS]))
                ins.sync_info = mybir.SyncInfo(
                    on_wait=waits[-_MAX_INST_WAITS:],
                    on_update=list(si.on_update or []),
                )
            out.append(ins)
        if changed:
            bb.instructions = out


class _PatchedTileContext(tile.TileContext):
    """Split the end-of-kernel drain's sem waits the same way (the drain is
    emitted after scheduling, outside _split_sync_waits' reach)."""

    def _drain_and_barrier(self, tick_clock, wait_clock):
        nc = self.nc
        drain_inst = nc.sync.drain()
        wait_clock.add_sem_waits(
            drain_inst.ins, ScopedClock({None: tick_clock.global_clock})
        )
        ins = drain_inst.ins
        si = ins.sync_info
        waits = list(si.on_wait or []) if si else []
        updates = list(si.on_update or []) if si else []
        if len(waits) > _MAX_DRAIN_WAITS:
            ins.sync_info = mybir.SyncInfo(
                on_wait=waits[:_MAX_DRAIN_WAITS], on_update=updates
            )
            rest = waits[_MAX_DRAIN_WAITS:]
            for i in range(0, len(rest), _MAX_DRAIN_WAITS):
                nop = nc.sync.nop(nofuse=True, hint=f"dw{i}")
                nop.ins.sync_info = mybir.SyncInfo(
                    on_wait=rest[i : i + _MAX_DRAIN_WAITS], on_update=[]
                )
        nc.all_engine_barrier()
        assert self.sems is not None
        popped = nc._tile_sem_poison_stack.pop()
        assert popped is self._sem_poison
        nc.clear_and_free_semaphores(list(self.sems.allocated().values()))
        nc.all_engine_barrier()


def _build(split_waits=True):
    nc = bass.Bass()

    # partition-major packed inputs (see _make_in_maps)
    xt_d = nc.dram_tensor("xt", [TQ, 2, 128, HC, 512], BF16, kind="ExternalInput")
    bqt_d = nc.dram_tensor("bqt", [128, DC, FQ], BF16, kind="ExternalInput")
    bkt_d = nc.dram_tensor("bkt", [128, DC, DH], BF16, kind="ExternalInput")
    bvt_d = nc.dram_tensor("bvt", [128, DC, DH], BF16, kind="ExternalInput")
    bot_d = nc.dram_tensor("bot", [8, 128, H, 512], BF16, kind="ExternalInput")
    sq_d = nc.dram_tensor("sq", [H, DH, 1], F32, kind="ExternalInput")
    sk_d = nc.dram_tensor("sk", [DH, 1], F32, kind="ExternalInput")
    sv_d = nc.dram_tensor("sv", [1, DH], F32, kind="ExternalInput")
    ones_d = nc.dram_tensor("ones", [DH, 1], BF16, kind="ExternalInput")
    y_d = nc.dram_tensor("y", [T, HIDDEN], BF16, kind="ExternalOutput")

    with _PatchedTileContext(nc) as tc:
        with (
            tc.tile_pool(name="wq", bufs=1) as wq,
            tc.tile_pool(name="wk", bufs=1) as wk,
            tc.tile_pool(name="wv", bufs=1) as wv,
            tc.tile_pool(name="xt", bufs=3) as xtp,
            tc.tile_pool(name="qt", bufs=H) as qtp,
            tc.tile_pool(name="kt", bufs=1) as ktp,
            tc.tile_pool(name="vv", bufs=TQ) as vvp,
            tc.tile_pool(name="pt", bufs=10) as ptp,
            tc.tile_pool(name="ot", bufs=H) as otp,
            tc.tile_pool(name="wo", bufs=2) as wop,
            tc.tile_pool(name="ys", bufs=4) as ysp,
            tc.tile_pool(name="sc", bufs=2) as scp,
            tc.tile_pool(name="misc", bufs=2) as misc,
            tc.tile_pool(name="psM", bufs=2, space="PSUM") as psM,
            tc.tile_pool(name="psS", bufs=2, space="PSUM") as psS,
            tc.tile_pool(name="psE", bufs=1, space="PSUM") as psE,
        ):
            # --- xt first half of tq0 first so compute starts ASAP -------
            xt_sb = {}  # (tq, half) -> [128, HC, 512] tile

            def load_xt(tq, half):
                t_ = xtp.tile([128, HC, 512], BF16, tag="xt",
                              name=f"xt{tq}_{half}")
                nc.sync.dma_start(t_[:], xt_d[tq, half])
                xt_sb[(tq, half)] = t_

            def xt_chunk(tq, dc):
                return xt_sb[(tq, dc // HC)][:, dc % HC, :]

            load_xt(0, 0)
            bkt_sb = wk.tile([128, DC, DH], BF16, tag="wk")
            nc.sync.dma_start(bkt_sb[:], bkt_d[:])
            load_xt(0, 1)
            bqt_sb = wq.tile([128, DC, FQ], BF16, tag="wq")
            nc.sync.dma_start(bqt_sb[:], bqt_d[:])
            bvt_sb = wv.tile([128, DC, DH], BF16, tag="wv")
            nc.sync.dma_start(bvt_sb[:], bvt_d[:])

            # --- constants / scales -------------------------------------
            sq_sb = [misc.tile([DH, 1], F32, tag=f"sq{f}", name=f"sq{f}")
                     for f in range(H)]
            for f in range(H):
                nc.sync.dma_start(sq_sb[f][:], sq_d[f])
            sk_sb = misc.tile([DH, 1], F32, tag="sk")
            nc.sync.dma_start(sk_sb[:], sk_d[:])
            sv_sb = misc.tile([1, DH], F32, tag="sv")
            nc.sync.dma_start(sv_sb[:], sv_d[:])
            ones_sb = misc.tile([DH, 1], BF16, tag="ones")
            nc.sync.dma_start(ones_sb[:], ones_d[:])

            # --- persistent activation tiles ----------------------------
            qt_sb = [qtp.tile([DH, T], BF16, tag="qt", name=f"qt{f}")
                     for f in range(H)]
            kt_sb = ktp.tile([DH, T], BF16, tag="kt")
            vv_sb = [vvp.tile([128, 512], BF16, tag="vv", name=f"vv{tq}")
                     for tq in range(TQ)]
            ot_sb = [otp.tile([DH, T], BF16, tag="ot", name=f"ot{f}")
                     for f in range(H)]

            # --- phase 1: q/k/v projections, one token-quarter at a time
            for tq in range(TQ):
                if tq > 0:
                    load_xt(tq, 0)
                    load_xt(tq, 1)
                tsl = slice(tq * 512, (tq + 1) * 512)

                # K projection: Kt[:, tq] += BkT_chunk^T @ xT_chunk
                ps = psM.tile([128, 512], F32, tag="mm", name=f"psk{tq}")
                for dc in range(DC):
                    nc.tensor.matmul(
                        ps[:], bkt_sb[:, dc, :], xt_chunk(tq, dc),
                        start=(dc == 0), stop=(dc == DC - 1),
                    )
                nc.scalar.activation(
                    kt_sb[:, tsl], ps[:],
                    mybir.ActivationFunctionType.Copy, scale=sk_sb[:],
                )

                # Q projection: Qt[f][:, tq] += BqT_chunk[:, f]^T @ xT_chunk
                for f in range(H):
                    ps = psM.tile([128, 512], F32, tag="mm",
                                  name=f"psq{tq}_{f}")
                    for dc in range(DC):
                        nc.tensor.matmul(
                            ps[:],
                            bqt_sb[:, dc, f * 128 : (f + 1) * 128],
                            xt_chunk(tq, dc),
                            start=(dc == 0), stop=(dc == DC - 1),
                        )
                    nc.scalar.activation(
                        qt_sb[f][:, tsl], ps[:],
                        mybir.ActivationFunctionType.Copy, scale=sq_sb[f][:],
                    )

                # V projection: V[tq][:, vt*128+d] (tokens on partitions)
                ps = psM.tile([128, 512], F32, tag="mm", name=f"psv{tq}")
                for vt in range(4):
                    for dc in range(DC):
                        nc.tensor.matmul(
                            ps[:, vt * 128 : (vt + 1) * 128],
                            xt_chunk(tq, dc)[:, vt * 128 : (vt + 1) * 128],
                            bvt_sb[:, dc, :],
                            start=(dc == 0), stop=(dc == DC - 1),
                        )
                nc.vector.tensor_copy(out=vv_sb[tq][:], in_=ps[:])

            # --- phase 2: attention per (head, query-block) --------------
            for h in range(H):
                for qb in range(QB):
                    qsl = slice(qb * 512, (qb + 1) * 512)
                    # scores (transposed) + exp, two key tiles per PSUM pair
                    pt_sb = []
                    for kp in range(KT // 2):
                        ps_s = psS.tile([128, 1024], F32, tag="s2",
                                        name=f"pss{h}_{qb}_{kp}")
                        for j in range(2):
                            kt = 2 * kp + j
                            nc.tensor.matmul(
                                ps_s[:, j * 512 : (j + 1) * 512],
                                kt_sb[:, kt * 128 : (kt + 1) * 128],
                                qt_sb[h][:, qsl],
                                start=True, stop=True,
                            )
                        pt = ptp.tile([128, 1024], BF16, tag="pt",
                                      name=f"pt{h}_{qb}_{kp}")
                        nc.scalar.activation(
                            pt[:], ps_s[:], mybir.ActivationFunctionType.Exp
                        )
                        pt_sb.append(pt)
                    # O.T = V^T @ P.T over key tiles; E = ones^T @ P.T
                    ps_o = psM.tile([128, 512], F32, tag="mm",
                                    name=f"pso{h}_{qb}")
                    ps_e = psE.tile([1, 512], F32, tag="e", name=f"pse{h}_{qb}")
                    for kt in range(KT):
                        tqi, vti = divmod(kt, 4)
                        rhs = pt_sb[kt // 2][:, (kt % 2) * 512 : (kt % 2 + 1) * 512]
                        nc.tensor.matmul(
                            ps_o[:],
                            vv_sb[tqi][:, vti * 128 : (vti + 1) * 128],
                            rhs,
                            start=(kt == 0), stop=(kt == KT - 1),
                        )
                        nc.tensor.matmul(
                            ps_e[:], ones_sb[:], rhs,
                            start=(kt == 0), stop=(kt == KT - 1),
                        )
                    recip = misc.tile([1, 512], F32, tag="recip",
                                      name=f"recip{h}_{qb}")
                    nc.vector.reciprocal(recip[:], ps_e[:])
                    ps_sc = psE.tile([128, 512], F32, tag="sc",
                                     name=f"pssc{h}_{qb}")
                    nc.tensor.matmul(
                        ps_sc[:], sv_sb[:], recip[:], start=True, stop=True
                    )
                    sc_sb = scp.tile([128, 512], F32, tag="sc",
                                     name=f"sc{h}_{qb}")
                    nc.scalar.activation(
                        sc_sb[:], ps_sc[:], mybir.ActivationFunctionType.Copy
                    )
                    nc.vector.tensor_tensor(
                        ot_sb[h][:, qsl], ps_o[:], sc_sb[:],
                        mybir.AluOpType.mult,
                    )

            # --- phase 3: o_proj partial --------------------------------
            for ob in range(8):
                osl = slice(ob * 512, (ob + 1) * 512)
                bot_sb = wop.tile([128, H, 512], BF16, tag="wo", name=f"wo{ob}")
                nc.sync.dma_start(bot_sb[:], bot_d[ob])
                for tt in range(16):
                    ps_y = psM.tile([128, 512], F32, tag="mm",
                                    name=f"psy{ob}_{tt}")
                    for c in range(H):
                        nc.tensor.matmul(
                            ps_y[:],
                            ot_sb[c][:, tt * 128 : (tt + 1) * 128],
                            bot_sb[:, c, :],
                            start=(c == 0), stop=(c == H - 1),
                        )
                    ysb = ysp.tile([128, 512], BF16, tag="ys",
                                   name=f"ys{ob}_{tt}")
                    nc.any.tensor_copy(out=ysb[:], in_=ps_y[:])
                    nc.sync.dma_start(
                        y_d[tt * 128 : (tt + 1) * 128, osl], ysb[:]
                    )

    if split_waits:
        _split_sync_waits(nc)
    return nc


_NC_CACHE = None


def _get_nc():
    global _NC_CACHE
    if _NC_CACHE is None:
        _NC_CACHE = _build()
    return _NC_CACHE


def _binarize(w):
    """Match reference bitnet_linear: s = max(mean|W|_row, 1e-8) (>0), so
    sign(W/s) == sign(W). Returns (sign(W) as bf16, s as f32)."""
    w = np.asarray(w, np.float32)
    s = np.maximum(
        np.abs(w).mean(axis=1, dtype=np.float64).astype(np.float32), 1e-8
    )
    return np.sign(w).astype(ml_dtypes.bfloat16), s


def _make_in_maps(hidden_states, q_weight, q_scale, k_weight, k_scale,
                  v_weight, v_scale, o_weight, o_scale):
    hs = np.asarray(hidden_states, np.float32)
    b, t, hid = hs.shape
    assert (b, t, hid) == (1, T, HIDDEN)

    xT = np.ascontiguousarray(hs[0].T).astype(ml_dtypes.bfloat16)
    # [d, t] -> [tq, half, p, c_in_half, f]   (d = (half*HC + c)*128 + p,
    #                                          t = tq*512 + f)
    xt4 = np.ascontiguousarray(
        xT.reshape(2, HC, 128, TQ, 512).transpose(3, 0, 2, 1, 4)
    )

    bq, s_q = _binarize(q_weight)
    bk, s_k = _binarize(k_weight)
    bv, s_v = _binarize(v_weight)
    bo, s_o = _binarize(o_weight)

    sq_full = s_q * np.asarray(q_scale, np.float32)                # [4096]
    sk_full = s_k * np.asarray(k_scale, np.float32) / np.sqrt(DH)  # [1024]
    sv_full = s_v * np.asarray(v_scale, np.float32)                # [1024]
    so_full = s_o * np.asarray(o_scale, np.float32)                # [4096]

    ones = np.ones((DH, 1), ml_dtypes.bfloat16)

    def pack_w(wt, nf):
        # [d, nf] -> [p, c, nf]
        return np.ascontiguousarray(wt.reshape(DC, 128, nf).transpose(1, 0, 2))

    in_maps = []
    for i in range(N_CORES):
        fq = slice(FQ * i, FQ * (i + 1))
        fk = slice(DH * i, DH * (i + 1))
        bot = np.ascontiguousarray(bo[:, fq].T)  # [512 cfeat, 4096 o]
        in_maps.append({
            "xt": xt4,
            "bqt": pack_w(np.ascontiguousarray(bq[fq].T), FQ),
            "bkt": pack_w(np.ascontiguousarray(bk[fk].T), DH),
            "bvt": pack_w(np.ascontiguousarray(bv[fk].T), DH),
            "bot": np.ascontiguousarray(
                bot.reshape(H, 128, 8, 512).transpose(2, 1, 0, 3)
            ),
            "sq": np.ascontiguousarray(
                sq_full[fq].reshape(H, DH, 1).astype(np.float32)
            ),
            "sk": np.ascontiguousarray(
                sk_full[fk].reshape(DH, 1).astype(np.float32)
            ),
            "sv": np.ascontiguousarray(
                sv_full[fk].reshape(1, DH).astype(np.float32)
            ),
            "ones": ones,
        })
    return in_maps, so_full


def kernel(**inputs):
    in_maps, so_full = _make_in_maps(**inputs)
    nc = _get_nc()
    res = run_bass_kernel_spmd(
        nc, in_maps, core_ids=list(range(N_CORES)), trace=TRACE
    )
    if TRACE:
        kernel.last_exec_time_ns = res.exec_time_ns
        kernel.last_mean_exec_time_ns = res.mean_exec_time_ns

    y = np.zeros((T, HIDDEN), np.float32)
    for i in range(N_CORES):
        y += res.results[i]["y"].astype(np.float32)
    y *= so_full[None, :]
    return y.reshape(1, T, HIDDEN)
